# revision 49
# baseline (speedup 1.0000x reference)
"""EnsembleTransitionMLP Trainium2 kernel.

Problem: 50-member ensemble of 4-layer MLPs (40 -> 256 -> 256 -> 256 -> 33),
shared input batch [8192, 40], fp32.

Sharding (8 cores): hybrid expert+batch. Work = 50 members x 4 batch-chunks
of 2048 = 200 units; each core gets 25 units: 6 full members (4 chunks each)
plus one quarter of a "leftover" member (members 48/49 are split 4-ways by
batch). Every core runs an identical instruction stream; per-core data
(weight slots, leftover batch slice) differs only in the input maps.

On-chip mapping: activations live as H^T [hidden on partitions, batch on
free], so weights load directly as lhsT=[K, M] (natural [in, out] layout)
and no transposes are ever needed. Host transposes SA once and the final
output once.

Matmul dtype: float32r (full fp32 bits; PE runs it at 1 cycle/row for
moving-dim >= 256, vs 4 cycles/row for plain fp32).
"""

import os
import sys
from contextlib import ExitStack

import numpy as np

import concourse.bass as bass
import concourse.tile as tile
from concourse import bacc, mybir
from concourse.bass_utils import run_bass_kernel_spmd

# ---------------------------------------------------------------- constants
CORES = 8
E = 50
B = 8192
IN_DIM = 40  # state 32 + action 8
H = 256
OD = 33  # next_state 32 + reward 1
SLOTS = 7  # per-core weight slots: 6 full members + 1 leftover member
CHUNK = 2048  # batch columns per unit
NT = CHUNK // 512  # 512-column N-tiles per unit
UNITS = 25  # 24 regular (slot u//4, chunk u%4) + 1 leftover (slot 6)

F32 = mybir.dt.float32
F32R = mybir.dt.float32r

# packed per-slot weight blob: per partition p ->
#   [ W2[p,:] | W2[128+p,:] | W3[p,:] | W3[128+p,:] |
#     W4[p,:] | W4[128+p,:] | W1[p,:] (p<40, else 0) ]
W2_OFF = 0
W3_OFF = 2 * H  # 512
W4_OFF = 4 * H  # 1024
W1_OFF = 4 * H + 2 * OD  # 1090
WCOLS = W1_OFF + H  # 1346

# 'f16' = fp16 in / fp32 accumulate (10-bit mantissa, FWL weight loads)
# 'f32r' = TF32 (10-bit mantissa, slow fp32 weight loads)
# 'f32'  = exact fp32, 4x slower PE | 'bf16' = fastest loads, 8-bit mantissa
MM_MODE = os.environ.get("MLP_MM_MODE", "f16")


def _mm(ap):
    return ap


# ---------------------------------------------------------------- program
def build_program():
    nc = bacc.Bacc(
        "TRN2",
        target_bir_lowering=False,
        debug=False,
        num_devices=CORES,
    )
    if MM_MODE == "bf16":
        mmdt = mybir.dt.bfloat16
    elif MM_MODE == "f16":
        mmdt = mybir.dt.float16
    elif MM_MODE == "f32r":
        mmdt = F32R
    else:
        mmdt = F32

    sat_d = nc.dram_tensor("sat", [IN_DIM, B], mmdt, kind="ExternalInput").ap()
    satx_d = nc.dram_tensor("satx", [IN_DIM, CHUNK], mmdt, kind="ExternalInput").ap()
    wpk_d = nc.dram_tensor("wpk", [SLOTS, 128, WCOLS], mmdt, kind="ExternalInput").ap()
    b1_d = nc.dram_tensor("b1", [SLOTS, H], F32, kind="ExternalInput").ap()
    b2_d = nc.dram_tensor("b2", [SLOTS, H], F32, kind="ExternalInput").ap()
    b3_d = nc.dram_tensor("b3", [SLOTS, H], F32, kind="ExternalInput").ap()
    b4_d = nc.dram_tensor("b4", [SLOTS, 128], F32, kind="ExternalInput").ap()
    out_d = nc.dram_tensor("out", [UNITS, OD, CHUNK], F32, kind="ExternalOutput").ap()

    with tile.TileContext(nc) as tc, ExitStack() as ctx:
        wpool = ctx.enter_context(tc.tile_pool(name="wpool", bufs=1))
        spool = ctx.enter_context(tc.tile_pool(name="spool", bufs=1))
        hpool = ctx.enter_context(tc.tile_pool(name="hpool", bufs=2))
        opool = ctx.enter_context(tc.tile_pool(name="opool", bufs=4))
        ppool = ctx.enter_context(tc.tile_pool(name="ppool", bufs=4, space="PSUM"))

        # ---- resident inputs -------------------------------------------
        # order: sat chunk0, slot-0 weights, biases, satx, remaining sat
        # chunks, remaining slots -> unit 0 never waits on the long tail.
        sat_t = spool.tile([IN_DIM, B], mmdt, name="sat_t")
        nc.sync.dma_start(out=sat_t[:, 0:CHUNK], in_=sat_d[:, 0:CHUNK])

        wpt = []
        for s in range(SLOTS):
            wp = wpool.tile([128, WCOLS], mmdt, name=f"wp_{s}", tag=f"wp_{s}")
            wpt.append(wp)
        nc.sync.dma_start(out=wpt[0][:, :], in_=wpk_d[0, :, :])

        # biases: one DMA per layer for all slots, [128, slots, mtile]
        b1a = wpool.tile([128, SLOTS, 2], F32, name="b1a", tag="b1a")
        nc.sync.dma_start(out=b1a[:, :, :], in_=b1_d.rearrange("s (m p) -> p s m", p=128))
        b2a = wpool.tile([128, SLOTS, 2], F32, name="b2a", tag="b2a")
        nc.sync.dma_start(out=b2a[:, :, :], in_=b2_d.rearrange("s (m p) -> p s m", p=128))
        b3a = wpool.tile([128, SLOTS, 2], F32, name="b3a", tag="b3a")
        nc.sync.dma_start(out=b3a[:, :, :], in_=b3_d.rearrange("s (m p) -> p s m", p=128))
        # b4 replicated at partitions 0-32 and 64-96 (host packs b4p) so
        # the two stacked L4 output blocks read bias from their own lanes
        b4a = wpool.tile([128, SLOTS], F32, name="b4a", tag="b4a")
        nc.sync.dma_start(out=b4a[:, :], in_=b4_d.rearrange("s p -> p s"))
        b1t = [b1a[:, s, :] for s in range(SLOTS)]
        b2t = [b2a[:, s, :] for s in range(SLOTS)]
        b3t = [b3a[:, s, :] for s in range(SLOTS)]

        # remaining inputs, in first-use order
        satx_t = spool.tile([IN_DIM, CHUNK], mmdt, name="satx_t")
        for c in range(1, 4):
            nc.sync.dma_start(
                out=sat_t[:, c * CHUNK : (c + 1) * CHUNK],
                in_=sat_d[:, c * CHUNK : (c + 1) * CHUNK],
            )
        for s in range(1, SLOTS):
            nc.sync.dma_start(out=wpt[s][:, :], in_=wpk_d[s, :, :])
        nc.sync.dma_start(out=satx_t[:, :], in_=satx_d[:, :])

        # weight slice views into the packed per-slot blob
        w1t = [wp[0:IN_DIM, W1_OFF : W1_OFF + H] for wp in wpt]
        w2t = [wp[:, W2_OFF : W2_OFF + 2 * H] for wp in wpt]
        w3t = [wp[:, W3_OFF : W3_OFF + 2 * H] for wp in wpt]
        w4t = [wp[:, W4_OFF : W4_OFF + 2 * OD] for wp in wpt]

        # ---- fused bias + relu (PSUM -> SBUF), cost-balanced ACT / DVE -
        eng_cost = [0.0, 0.0]  # ACT, DVE accumulated ns

        def bias_relu(dst, src, bias_ap, relu):
            n = src.free_size()
            act_cost = n / 1.2 + 260
            dve_cost = n / 0.96 + 215
            if eng_cost[0] + act_cost <= eng_cost[1] + dve_cost:
                eng_cost[0] += act_cost
                func = (
                    mybir.ActivationFunctionType.Relu
                    if relu
                    else mybir.ActivationFunctionType.Identity
                )
                nc.scalar.activation(dst, src, func, bias=bias_ap, scale=1.0)
            else:
                eng_cost[1] += dve_cost
                if relu:
                    nc.vector.tensor_scalar(
                        dst, src, bias_ap, 0.0, mybir.AluOpType.add, mybir.AluOpType.max
                    )
                else:
                    nc.vector.tensor_scalar(
                        dst, src, bias_ap, None, mybir.AluOpType.add
                    )

        # ---- compute: 25 units, layers interleaved across unit groups --
        # PSUM tiles are [128, 1024] (2 banks); matmuls fill 512-col halves
        # (each within one bank), bias+relu drains 1024 cols per op.
        # Units run in groups; within a group each layer of unit a is
        # followed by the same layer of unit b, so PE chews b's matmuls
        # while a's relu chain drains -> no layer-boundary stalls.
        h_cur = {}  # unit -> h tile of previous layer

        def emit_l1_half(u, tp):
            s = u // 4 if u < 24 else 6
            src = sat_t if u < 24 else satx_t
            c0 = (u % 4) * CHUNK if u < 24 else 0
            if tp == 0:
                h_cur[u] = hpool.tile([128, 2, CHUNK], mmdt, name="h1", tag="h1")
            h1 = h_cur[u]
            for m in range(2):
                ps = ppool.tile([128, 1024], F32, name="ps", tag="ps")
                for th in range(2):
                    t = 2 * tp + th
                    nc.tensor.matmul(
                        out=ps[:, th * 512 : (th + 1) * 512],
                        lhsT=w1t[s][:, m * 128 : (m + 1) * 128],
                        rhs=src[:, c0 + t * 512 : c0 + (t + 1) * 512],
                        start=True,
                        stop=True,
                    )
                bias_relu(
                    h1[:, m, tp * 1024 : (tp + 1) * 1024],
                    ps[:, :],
                    b1t[s][:, m : m + 1],
                    True,
                )

        def emit_l1(u):
            for tp in range(NT // 2):
                emit_l1_half(u, tp)

        def emit_mid(u, w, bt, nm):
            # k outer: k0 matmuls read h_prev[mtile 0] (earliest relus),
            # k1 read h_prev[mtile 1] (latest) -> matches relu finish order.
            s = u // 4 if u < 24 else 6
            h_prev = h_cur[u]
            h_nxt = hpool.tile([128, 2, CHUNK], mmdt, name=nm, tag=nm)
            for tp in range(NT // 2):
                for m in range(2):
                    ps = ppool.tile([128, 1024], F32, name="ps", tag="ps")
                    for k in range(2):
                        for th in range(2):
                            t = 2 * tp + th
                            nsl = slice(t * 512, (t + 1) * 512)
                            nc.tensor.matmul(
                                out=ps[:, th * 512 : (th + 1) * 512],
                                lhsT=w[s][:, k * H + m * 128 : k * H + (m + 1) * 128],
                                rhs=h_prev[:, k, nsl],
                                start=(k == 0),
                                stop=(k == 1),
                            )
                    bias_relu(
                        h_nxt[:, m, tp * 1024 : (tp + 1) * 1024],
                        ps[:, :],
                        bt[s][:, m : m + 1],
                        True,
                    )
            h_cur[u] = h_nxt

        def emit_l4(u):
            # both tp-pairs stack into ONE PSUM tile at partition offsets
            # 0 / 64 (legal tile positions for a 33-row output): one slot
            # per unit instead of two, and two wide 1024-col drains.
            s = u // 4 if u < 24 else 6
            h_prev = h_cur[u]
            ot = opool.tile([128, 1024], F32, name="ot", tag="ot")
            ps = ppool.tile([128, 1024], F32, name="ps", tag="ps")
            for tp in range(NT // 2):
                po = 64 * tp
                for k in range(2):
                    for th in range(2):
                        t = 2 * tp + th
                        nsl = slice(t * 512, (t + 1) * 512)
                        nc.tensor.matmul(
                            out=ps[po : po + OD, th * 512 : (th + 1) * 512],
                            lhsT=w4t[s][:, k * OD : (k + 1) * OD],
                            rhs=h_prev[:, k, nsl],
                            start=(k == 0),
                            stop=(k == 1),
                        )
                bias_relu(
                    ot[po : po + OD, :],
                    ps[po : po + OD, :],
                    b4a[po : po + OD, s : s + 1],
                    False,
                )
                nc.sync.dma_start(
                    out=out_d[u, :, tp * 1024 : (tp + 1) * 1024],
                    in_=ot[po : po + OD, :],
                )
            del h_cur[u]

        # software-pipelined: L1 of unit u+1 is emitted between L2(u) and
        # L3(u), so every layer boundary has >=1 layer of independent
        # matmuls in flight while the producing relu chain drains.
        # L1(u+1) emitted in two halves straddling L4(u): covers both the
        # L3->L4 and L4->L2(u+1) boundaries and halves L1's PSUM burst.
        emit_l1(0)
        for u in range(UNITS):
            emit_mid(u, w2t, b2t, "h2")
            emit_mid(u, w3t, b3t, "h3")
            if u + 1 < UNITS:
                emit_l1_half(u + 1, 0)
            emit_l4(u)
            if u + 1 < UNITS:
                emit_l1_half(u + 1, 1)

    nc.compile()
    return nc


# ---------------------------------------------------------------- host side
def _cast(a):
    if MM_MODE == "bf16":
        import ml_dtypes

        return np.asarray(a, dtype=ml_dtypes.bfloat16)
    if MM_MODE == "f16":
        return np.ascontiguousarray(np.asarray(a, dtype=np.float32).astype(np.float16))
    return np.ascontiguousarray(a, dtype=np.float32)


def _pack_weights(W1, W2, W3, W4, members):
    """[len(members), 128, WCOLS] packed per-slot blob (see layout above)."""
    n = len(members)
    dt = _cast(np.zeros(1)).dtype
    wpk = np.zeros((n, 128, WCOLS), dtype=dt)
    for i, m in enumerate(members):
        wpk[i, :, W2_OFF : W2_OFF + H] = W2[m, :128, :]
        wpk[i, :, W2_OFF + H : W2_OFF + 2 * H] = W2[m, 128:, :]
        wpk[i, :, W3_OFF : W3_OFF + H] = W3[m, :128, :]
        wpk[i, :, W3_OFF + H : W3_OFF + 2 * H] = W3[m, 128:, :]
        wpk[i, :, W4_OFF : W4_OFF + OD] = W4[m, :128, :]
        wpk[i, :, W4_OFF + OD : W4_OFF + 2 * OD] = W4[m, 128:, :]
        wpk[i, :IN_DIM, W1_OFF : W1_OFF + H] = W1[m]
    return wpk


def _pack_b4(b4, members):
    """[slots, 128] with b4 replicated at partitions 0-32 and 64-96."""
    out = np.zeros((len(members), 128), np.float32)
    out[:, :OD] = b4[members]
    out[:, 64 : 64 + OD] = b4[members]
    return out


def make_in_maps(inputs):
    sa = np.concatenate(
        [np.asarray(inputs["state"]), np.asarray(inputs["action"])], axis=1
    )
    sat = _cast(sa.T)  # [40, 8192]
    W1 = _cast(np.asarray(inputs["W1"]))
    W2 = _cast(np.asarray(inputs["W2"]))
    W3 = _cast(np.asarray(inputs["W3"]))
    W4 = _cast(np.asarray(inputs["W4"]))
    in_maps = []
    for k in range(CORES):
        members = list(range(k * 6, (k + 1) * 6)) + [48 + k // 4]
        lc = k % 4  # leftover member's batch chunk handled by this core
        im = {
            "sat": sat,
            "satx": _cast(sat[:, lc * CHUNK : (lc + 1) * CHUNK]),
            "wpk": _pack_weights(W1, W2, W3, W4, members),
            "b1": np.ascontiguousarray(np.asarray(inputs["b1"])[members], np.float32),
            "b2": np.ascontiguousarray(np.asarray(inputs["b2"])[members], np.float32),
            "b3": np.ascontiguousarray(np.asarray(inputs["b3"])[members], np.float32),
            "b4": _pack_b4(np.asarray(inputs["b4"]), members),
        }
        in_maps.append(im)
    return in_maps


def assemble(results):
    predsT = np.empty((E, OD, B), np.float32)
    for k in range(CORES):
        o = results[k]["out"]  # [25, 33, 2048]
        for u in range(24):
            s, c = divmod(u, 4)
            predsT[k * 6 + s, :, c * CHUNK : (c + 1) * CHUNK] = o[u]
        m = 48 + k // 4
        c = k % 4
        predsT[m, :, c * CHUNK : (c + 1) * CHUNK] = o[24]
    preds = predsT.transpose(2, 0, 1)  # [B, E, 33]
    return np.ascontiguousarray(preds[..., :-1]), np.ascontiguousarray(preds[..., -1])


_NC = None


def _get_nc():
    global _NC
    if _NC is None:
        _NC = build_program()
    return _NC


def run(inputs, trace=False, **kw):
    nc = _get_nc()
    in_maps = make_in_maps(inputs)
    res = run_bass_kernel_spmd(nc, in_maps, list(range(CORES)), trace=trace, **kw)
    return assemble(res.results), res


def kernel(**inputs):
    (next_state, reward), _ = run(inputs, trace=False)
    return next_state, reward


# revision 50
# speedup vs baseline: 1.0232x; 1.0232x over previous
"""EnsembleTransitionMLP Trainium2 kernel.

Problem: 50-member ensemble of 4-layer MLPs (40 -> 256 -> 256 -> 256 -> 33),
shared input batch [8192, 40], fp32.

Sharding (8 cores): hybrid expert+batch. Work = 50 members x 4 batch-chunks
of 2048 = 200 units; each core gets 25 units: 6 full members (4 chunks each)
plus one quarter of a "leftover" member (members 48/49 are split 4-ways by
batch). Every core runs an identical instruction stream; per-core data
(weight slots, leftover batch slice) differs only in the input maps.

On-chip mapping: activations live as H^T [hidden on partitions, batch on
free], so weights load directly as lhsT=[K, M] (natural [in, out] layout)
and no transposes are ever needed. Host transposes SA once and the final
output once.

Matmul dtype: float32r (full fp32 bits; PE runs it at 1 cycle/row for
moving-dim >= 256, vs 4 cycles/row for plain fp32).
"""

import os
import sys
from contextlib import ExitStack

import numpy as np

import concourse.bass as bass
import concourse.tile as tile
from concourse import bacc, mybir
from concourse.bass_utils import run_bass_kernel_spmd

# ---------------------------------------------------------------- constants
CORES = 8
E = 50
B = 8192
IN_DIM = 40  # state 32 + action 8
H = 256
OD = 33  # next_state 32 + reward 1
SLOTS = 7  # per-core weight slots: 6 full members + 1 leftover member
CHUNK = 2048  # batch columns per unit
NT = CHUNK // 512  # 512-column N-tiles per unit
UNITS = 25  # 24 regular (slot u//4, chunk u%4) + 1 leftover (slot 6)

F32 = mybir.dt.float32
F32R = mybir.dt.float32r

# packed per-slot weight blob: per partition p ->
#   [ W2[p,:] | W2[128+p,:] | W3[p,:] | W3[128+p,:] |
#     W4[p,:] | W4[128+p,:] | W1[p,:] (p<40, else 0) ]
W2_OFF = 0
W3_OFF = 2 * H  # 512
W4_OFF = 4 * H  # 1024
W1_OFF = 4 * H + 2 * OD  # 1090
WCOLS = W1_OFF + H  # 1346

# 'f16' = fp16 in / fp32 accumulate (10-bit mantissa, FWL weight loads)
# 'f32r' = TF32 (10-bit mantissa, slow fp32 weight loads)
# 'f32'  = exact fp32, 4x slower PE | 'bf16' = fastest loads, 8-bit mantissa
MM_MODE = os.environ.get("MLP_MM_MODE", "f16")


def _mm(ap):
    return ap


# ---------------------------------------------------------------- program
def build_program():
    nc = bacc.Bacc(
        "TRN2",
        target_bir_lowering=False,
        debug=False,
        num_devices=CORES,
    )
    if MM_MODE == "bf16":
        mmdt = mybir.dt.bfloat16
    elif MM_MODE == "f16":
        mmdt = mybir.dt.float16
    elif MM_MODE == "f32r":
        mmdt = F32R
    else:
        mmdt = F32

    sat_d = nc.dram_tensor("sat", [IN_DIM, B], mmdt, kind="ExternalInput").ap()
    satx_d = nc.dram_tensor("satx", [IN_DIM, CHUNK], mmdt, kind="ExternalInput").ap()
    wpk_d = nc.dram_tensor("wpk", [SLOTS, 128, WCOLS], mmdt, kind="ExternalInput").ap()
    b1_d = nc.dram_tensor("b1", [SLOTS, H], F32, kind="ExternalInput").ap()
    b2_d = nc.dram_tensor("b2", [SLOTS, H], F32, kind="ExternalInput").ap()
    b3_d = nc.dram_tensor("b3", [SLOTS, H], F32, kind="ExternalInput").ap()
    b4_d = nc.dram_tensor("b4", [SLOTS, OD], F32, kind="ExternalInput").ap()
    out_d = nc.dram_tensor("out", [UNITS, OD, CHUNK], F32, kind="ExternalOutput").ap()

    with tile.TileContext(nc) as tc, ExitStack() as ctx:
        wpool = ctx.enter_context(tc.tile_pool(name="wpool", bufs=1))
        spool = ctx.enter_context(tc.tile_pool(name="spool", bufs=1))
        hpool = ctx.enter_context(tc.tile_pool(name="hpool", bufs=2))
        opool = ctx.enter_context(tc.tile_pool(name="opool", bufs=4))
        ppool = ctx.enter_context(tc.tile_pool(name="ppool", bufs=4, space="PSUM"))

        # ---- resident inputs -------------------------------------------
        # order: sat chunk0, slot-0 weights, biases, satx, remaining sat
        # chunks, remaining slots -> unit 0 never waits on the long tail.
        sat_t = spool.tile([IN_DIM, B], mmdt, name="sat_t")
        nc.sync.dma_start(out=sat_t[:, 0:CHUNK], in_=sat_d[:, 0:CHUNK])

        wpt = []
        for s in range(SLOTS):
            wp = wpool.tile([128, WCOLS], mmdt, name=f"wp_{s}", tag=f"wp_{s}")
            wpt.append(wp)
        nc.sync.dma_start(out=wpt[0][:, :], in_=wpk_d[0, :, :])

        # biases: one DMA per layer for all slots, [128, slots, mtile]
        b1a = wpool.tile([128, SLOTS, 2], F32, name="b1a", tag="b1a")
        nc.sync.dma_start(out=b1a[:, :, :], in_=b1_d.rearrange("s (m p) -> p s m", p=128))
        b2a = wpool.tile([128, SLOTS, 2], F32, name="b2a", tag="b2a")
        nc.sync.dma_start(out=b2a[:, :, :], in_=b2_d.rearrange("s (m p) -> p s m", p=128))
        b3a = wpool.tile([128, SLOTS, 2], F32, name="b3a", tag="b3a")
        nc.sync.dma_start(out=b3a[:, :, :], in_=b3_d.rearrange("s (m p) -> p s m", p=128))
        b4a = wpool.tile([OD, SLOTS], F32, name="b4a", tag="b4a")
        nc.sync.dma_start(out=b4a[:, :], in_=b4_d.rearrange("s p -> p s"))
        b1t = [b1a[:, s, :] for s in range(SLOTS)]
        b2t = [b2a[:, s, :] for s in range(SLOTS)]
        b3t = [b3a[:, s, :] for s in range(SLOTS)]
        b4t = [b4a[:, s : s + 1] for s in range(SLOTS)]

        # remaining inputs, in first-use order
        satx_t = spool.tile([IN_DIM, CHUNK], mmdt, name="satx_t")
        for c in range(1, 4):
            nc.sync.dma_start(
                out=sat_t[:, c * CHUNK : (c + 1) * CHUNK],
                in_=sat_d[:, c * CHUNK : (c + 1) * CHUNK],
            )
        for s in range(1, SLOTS):
            nc.sync.dma_start(out=wpt[s][:, :], in_=wpk_d[s, :, :])
        nc.sync.dma_start(out=satx_t[:, :], in_=satx_d[:, :])

        # weight slice views into the packed per-slot blob
        w1t = [wp[0:IN_DIM, W1_OFF : W1_OFF + H] for wp in wpt]
        w2t = [wp[:, W2_OFF : W2_OFF + 2 * H] for wp in wpt]
        w3t = [wp[:, W3_OFF : W3_OFF + 2 * H] for wp in wpt]
        w4t = [wp[:, W4_OFF : W4_OFF + 2 * OD] for wp in wpt]

        # ---- fused bias + relu (PSUM -> SBUF), cost-balanced ACT / DVE -
        eng_cost = [0.0, 0.0]  # ACT, DVE accumulated ns

        def bias_relu(dst, src, bias_ap, relu):
            n = src.free_size()
            act_cost = n / 1.2 + 260
            dve_cost = n / 0.96 + 215
            if eng_cost[0] + act_cost <= eng_cost[1] + dve_cost:
                eng_cost[0] += act_cost
                func = (
                    mybir.ActivationFunctionType.Relu
                    if relu
                    else mybir.ActivationFunctionType.Identity
                )
                nc.scalar.activation(dst, src, func, bias=bias_ap, scale=1.0)
            else:
                eng_cost[1] += dve_cost
                if relu:
                    nc.vector.tensor_scalar(
                        dst, src, bias_ap, 0.0, mybir.AluOpType.add, mybir.AluOpType.max
                    )
                else:
                    nc.vector.tensor_scalar(
                        dst, src, bias_ap, None, mybir.AluOpType.add
                    )

        # ---- compute: 25 units, layers interleaved across unit groups --
        # PSUM tiles are [128, 1024] (2 banks); matmuls fill 512-col halves
        # (each within one bank), bias+relu drains 1024 cols per op.
        # Units run in groups; within a group each layer of unit a is
        # followed by the same layer of unit b, so PE chews b's matmuls
        # while a's relu chain drains -> no layer-boundary stalls.
        h_cur = {}  # unit -> h tile of previous layer

        def emit_l1_half(u, tp):
            s = u // 4 if u < 24 else 6
            src = sat_t if u < 24 else satx_t
            c0 = (u % 4) * CHUNK if u < 24 else 0
            if tp == 0:
                h_cur[u] = hpool.tile([128, 2, CHUNK], mmdt, name="h1", tag="h1")
            h1 = h_cur[u]
            for m in range(2):
                ps = ppool.tile([128, 1024], F32, name="ps", tag="ps")
                for th in range(2):
                    t = 2 * tp + th
                    nc.tensor.matmul(
                        out=ps[:, th * 512 : (th + 1) * 512],
                        lhsT=w1t[s][:, m * 128 : (m + 1) * 128],
                        rhs=src[:, c0 + t * 512 : c0 + (t + 1) * 512],
                        start=True,
                        stop=True,
                    )
                bias_relu(
                    h1[:, m, tp * 1024 : (tp + 1) * 1024],
                    ps[:, :],
                    b1t[s][:, m : m + 1],
                    True,
                )

        def emit_l1(u):
            for tp in range(NT // 2):
                emit_l1_half(u, tp)

        def emit_mid(u, w, bt, nm):
            # k outer: k0 matmuls read h_prev[mtile 0] (earliest relus),
            # k1 read h_prev[mtile 1] (latest) -> matches relu finish order.
            s = u // 4 if u < 24 else 6
            h_prev = h_cur[u]
            h_nxt = hpool.tile([128, 2, CHUNK], mmdt, name=nm, tag=nm)
            for tp in range(NT // 2):
                for m in range(2):
                    ps = ppool.tile([128, 1024], F32, name="ps", tag="ps")
                    for k in range(2):
                        for th in range(2):
                            t = 2 * tp + th
                            nsl = slice(t * 512, (t + 1) * 512)
                            nc.tensor.matmul(
                                out=ps[:, th * 512 : (th + 1) * 512],
                                lhsT=w[s][:, k * H + m * 128 : k * H + (m + 1) * 128],
                                rhs=h_prev[:, k, nsl],
                                start=(k == 0),
                                stop=(k == 1),
                            )
                    bias_relu(
                        h_nxt[:, m, tp * 1024 : (tp + 1) * 1024],
                        ps[:, :],
                        bt[s][:, m : m + 1],
                        True,
                    )
            h_cur[u] = h_nxt

        def emit_l4(u):
            s = u // 4 if u < 24 else 6
            h_prev = h_cur[u]
            ot = opool.tile([OD, CHUNK], F32, name="ot", tag="ot")
            for tp in range(NT // 2):
                ps = ppool.tile([128, 1024], F32, name="ps", tag="ps")
                for k in range(2):
                    for th in range(2):
                        t = 2 * tp + th
                        nsl = slice(t * 512, (t + 1) * 512)
                        nc.tensor.matmul(
                            out=ps[:OD, th * 512 : (th + 1) * 512],
                            lhsT=w4t[s][:, k * OD : (k + 1) * OD],
                            rhs=h_prev[:, k, nsl],
                            start=(k == 0),
                            stop=(k == 1),
                        )
                # 512-col drains on both engines free the PSUM slot faster
                for th in range(2):
                    bias_relu(
                        ot[:, tp * 1024 + th * 512 : tp * 1024 + (th + 1) * 512],
                        ps[:OD, th * 512 : (th + 1) * 512],
                        b4t[s][:, 0:1],
                        False,
                    )
            nc.sync.dma_start(out=out_d[u, :, :], in_=ot[:, :])
            del h_cur[u]

        # software-pipelined: L1 of unit u+1 is emitted between L2(u) and
        # L3(u), so every layer boundary has >=1 layer of independent
        # matmuls in flight while the producing relu chain drains.
        # L1(u+1) emitted in two halves straddling L4(u): covers both the
        # L3->L4 and L4->L2(u+1) boundaries and halves L1's PSUM burst.
        emit_l1(0)
        for u in range(UNITS):
            emit_mid(u, w2t, b2t, "h2")
            emit_mid(u, w3t, b3t, "h3")
            if u + 1 < UNITS:
                emit_l1_half(u + 1, 0)
            emit_l4(u)
            if u + 1 < UNITS:
                emit_l1_half(u + 1, 1)

    nc.compile()
    return nc


# ---------------------------------------------------------------- host side
def _cast(a):
    if MM_MODE == "bf16":
        import ml_dtypes

        return np.asarray(a, dtype=ml_dtypes.bfloat16)
    if MM_MODE == "f16":
        return np.ascontiguousarray(np.asarray(a, dtype=np.float32).astype(np.float16))
    return np.ascontiguousarray(a, dtype=np.float32)


def _pack_weights(W1, W2, W3, W4, members):
    """[len(members), 128, WCOLS] packed per-slot blob (see layout above)."""
    n = len(members)
    dt = _cast(np.zeros(1)).dtype
    wpk = np.zeros((n, 128, WCOLS), dtype=dt)
    for i, m in enumerate(members):
        wpk[i, :, W2_OFF : W2_OFF + H] = W2[m, :128, :]
        wpk[i, :, W2_OFF + H : W2_OFF + 2 * H] = W2[m, 128:, :]
        wpk[i, :, W3_OFF : W3_OFF + H] = W3[m, :128, :]
        wpk[i, :, W3_OFF + H : W3_OFF + 2 * H] = W3[m, 128:, :]
        wpk[i, :, W4_OFF : W4_OFF + OD] = W4[m, :128, :]
        wpk[i, :, W4_OFF + OD : W4_OFF + 2 * OD] = W4[m, 128:, :]
        wpk[i, :IN_DIM, W1_OFF : W1_OFF + H] = W1[m]
    return wpk


def make_in_maps(inputs):
    sa = np.concatenate(
        [np.asarray(inputs["state"]), np.asarray(inputs["action"])], axis=1
    )
    sat = _cast(sa.T)  # [40, 8192]
    W1 = _cast(np.asarray(inputs["W1"]))
    W2 = _cast(np.asarray(inputs["W2"]))
    W3 = _cast(np.asarray(inputs["W3"]))
    W4 = _cast(np.asarray(inputs["W4"]))
    in_maps = []
    for k in range(CORES):
        members = list(range(k * 6, (k + 1) * 6)) + [48 + k // 4]
        lc = k % 4  # leftover member's batch chunk handled by this core
        im = {
            "sat": sat,
            "satx": _cast(sat[:, lc * CHUNK : (lc + 1) * CHUNK]),
            "wpk": _pack_weights(W1, W2, W3, W4, members),
            "b1": np.ascontiguousarray(np.asarray(inputs["b1"])[members], np.float32),
            "b2": np.ascontiguousarray(np.asarray(inputs["b2"])[members], np.float32),
            "b3": np.ascontiguousarray(np.asarray(inputs["b3"])[members], np.float32),
            "b4": np.ascontiguousarray(np.asarray(inputs["b4"])[members], np.float32),
        }
        in_maps.append(im)
    return in_maps


def assemble(results):
    predsT = np.empty((E, OD, B), np.float32)
    for k in range(CORES):
        o = results[k]["out"]  # [25, 33, 2048]
        for u in range(24):
            s, c = divmod(u, 4)
            predsT[k * 6 + s, :, c * CHUNK : (c + 1) * CHUNK] = o[u]
        m = 48 + k // 4
        c = k % 4
        predsT[m, :, c * CHUNK : (c + 1) * CHUNK] = o[24]
    preds = predsT.transpose(2, 0, 1)  # [B, E, 33]
    return np.ascontiguousarray(preds[..., :-1]), np.ascontiguousarray(preds[..., -1])


_NC = None


def _get_nc():
    global _NC
    if _NC is None:
        _NC = build_program()
    return _NC


def run(inputs, trace=False, **kw):
    nc = _get_nc()
    in_maps = make_in_maps(inputs)
    res = run_bass_kernel_spmd(nc, in_maps, list(range(CORES)), trace=trace, **kw)
    return assemble(res.results), res


def kernel(**inputs):
    (next_state, reward), _ = run(inputs, trace=False)
    return next_state, reward
